# revision 12
# baseline (speedup 1.0000x reference)
"""DiT block kernel for 8 Trainium2 NeuronCores (Bass/Tile).

Sharding: sequence dim L=4096 split 8 ways (512 rows per core). Each core
computes K/V only for its OWN 512 rows, then an HBM AllGather distributes
the full K/V to every core (3 pipelined group collectives, 4 heads each).
The tiny AdaLN cond-projection GEMV is also sharded 8 ways and gathered.
Attention, out-projection and the FFN run on the local 512 rows only.
Weights are bf16; accumulation is fp32 in PSUM; layernorm statistics and
residuals are fp32. Softmax uses an augmented-V ones column for the
denominator (no max subtraction; logits are ~N(0,1) after 1/8 scaling).
"""

import sys

sys.path.insert(0, "/opt/trn_rl_repo")

from contextlib import ExitStack

import numpy as np
import ml_dtypes

import concourse.bass as bass
import concourse.bacc as bacc
import concourse.tile as tile
import concourse.mybir as mybir
from concourse.bass_utils import run_bass_kernel_spmd
from concourse.masks import make_identity

F32 = mybir.dt.float32
BF16 = mybir.dt.bfloat16
AF = mybir.ActivationFunctionType
OP = mybir.AluOpType

L, D, H, HD, DM = 4096, 768, 12, 64, 3072
NCORES = 8
LQ = L // NCORES  # 512 local rows
P = 128
EPS = 1e-5
NDC = D // P  # 6 chunks of the model dim
NHP = H // 2  # 6 head pairs
NMC = DM // P  # 24 chunks of the FFN hidden dim
NQC = LQ // P  # 4 local row chunks
NG = 3  # head groups for the K/V collective (4 heads each)
VW = HD + 1  # V channels per head incl ones column
GW = 4 * VW  # V channels per group (260)
CCW = 2 * LQ + 4 * GW  # collective width per group: kT(1024) + v(1040)

# AdaLN chunk ownership: 36 chunks of 128 over the concatenated
# [adaln1 | adaln2] output (2*2304 = 4608). Cores 0-3 own 5 chunks,
# cores 4-7 own 4 (+1 pad so the gather is uniform at 640 floats).
ADW = 640


def _ad_owner(k):
    if k < 20:
        return k // 5, k % 5
    return 4 + (k - 20) // 4, (k - 20) % 4


def _declare_params(nc):
    dp = nc.declare_dram_parameter
    t = {}
    t["x_loc"] = dp("x_loc", [LQ, D], F32, isOutput=False)
    t["cond_t"] = dp("cond_t", [P, NDC], F32, isOutput=False)
    t["w_ad_mine"] = dp("w_ad_mine", [P, NDC, ADW], BF16, isOutput=False)
    t["b_ad_mine"] = dp("b_ad_mine", [1, ADW], F32, isOutput=False)
    t["w_qkv"] = dp("w_qkv", [D, 3 * D], BF16, isOutput=False)
    t["bq_col"] = dp("bq_col", [P, NHP], F32, isOutput=False)
    t["bk_col"] = dp("bk_col", [P, NHP], F32, isOutput=False)
    t["bv_b"] = dp("bv_b", [P, D], F32, isOutput=False)
    t["w_attn_out"] = dp("w_attn_out", [D, D], BF16, isOutput=False)
    t["b_attn_b"] = dp("b_attn_b", [P, D], F32, isOutput=False)
    t["w_ffn1"] = dp("w_ffn1", [D, DM], BF16, isOutput=False)
    t["b_ffn1_col"] = dp("b_ffn1_col", [P, NMC], F32, isOutput=False)
    t["w_ffn2"] = dp("w_ffn2", [DM, D], BF16, isOutput=False)
    t["b_ffn2_b"] = dp("b_ffn2_b", [P, D], F32, isOutput=False)
    t["out"] = dp("out", [LQ, D], F32, isOutput=True)
    # DRAM scratch for the collectives
    t["dum_in"] = nc.dram_tensor("dum_in", [1, 2], F32)
    t["dum_out"] = nc.dram_tensor("dum_out", [NCORES, 2], F32, addr_space="Shared")
    t["ad_in"] = nc.dram_tensor("ad_in", [1, ADW], F32)
    t["ad_out"] = nc.dram_tensor("ad_out", [NCORES, ADW], F32, addr_space="Shared")
    t["kv_in"] = [nc.dram_tensor(f"kv_in{g}", [P, CCW], BF16) for g in range(NG)]
    t["kv_out"] = [
        nc.dram_tensor(f"kv_out{g}", [NCORES * P, CCW], BF16, addr_space="Shared")
        for g in range(NG)
    ]
    return t


def _ln_rstd(nc, spool, eps_t, xt, halves=2):
    """LayerNorm stats for a [P, D] f32 tile -> (mean [P,1], rstd [P,1]).

    rstd = exp(-0.5*ln(var+eps)) keeps the Act engine on the ln/exp table
    (no sqrt-table switch next to the softmax exp)."""
    v = nc.vector
    act = nc.scalar.activation
    stats = spool.tile([P, halves, 6], F32)
    w = D // halves
    for gki in range(halves):
        v.bn_stats(stats[:, gki, :], xt[:, gki * w : (gki + 1) * w])
    mv = spool.tile([P, 2], F32)
    v.bn_aggr(mv[:], stats[:])
    lnv = spool.tile([P, 1], F32)
    act(lnv[:], mv[:, 1:2], AF.Ln, bias=eps_t[:, 0:1])
    rstd = spool.tile([P, 1], F32)
    act(rstd[:], lnv[:], AF.Exp, scale=-0.5)
    return mv, rstd


def _build_body(nc, tc, ctx, t):
    mm = nc.tensor.matmul
    dma = nc.sync.dma_start
    v = nc.vector
    act = nc.scalar.activation
    rg = [list(range(NCORES))]

    # dependency-free warmup collective: absorbs the one-time NRT channel
    # setup latency while phase A/B compute runs
    nc.gpsimd.collective_compute(
        kind="AllGather",
        op=OP.bypass,
        replica_groups=rg,
        ins=[t["dum_in"][:]],
        outs=[t["dum_out"][:]],
    )

    const = ctx.enter_context(tc.tile_pool(name="const", bufs=1))
    identity = const.tile([P, P], BF16)
    make_identity(nc, identity)
    eps_t = const.tile([P, 1], F32)
    v.memset(eps_t, EPS)

    # Long-lived tiles
    big = ctx.enter_context(tc.tile_pool(name="big", bufs=1))
    colmat = big.tile([P, 36], F32)  # gathered adaln columns
    sp1 = big.tile([P, NDC], F32)  # 1 + scale1
    sp2 = big.tile([P, NDC], F32)
    g1_b = big.tile([P, D], F32)  # gate1 broadcast over partitions
    g2_b = big.tile([P, D], F32)
    qT_all = big.tile([P, NHP, LQ], BF16)
    catT_all = big.tile([P, NDC, LQ], BF16)  # attention output, transposed
    xn2T = big.tile([P, NDC, LQ], BF16)
    x2_loc = [big.tile([P, D], F32, name=f"x2_{q}") for q in range(NQC)]

    # ---------------- phase A: sharded AdaLN GEMV + gather ----------------
    with ExitStack() as phA:
        pool = phA.enter_context(tc.tile_pool(name="phA", bufs=1))
        psA = phA.enter_context(tc.tile_pool(name="psA", bufs=2, space="PSUM"))

        cond_sb = pool.tile([P, NDC], F32)
        dma(out=cond_sb[:], in_=t["cond_t"][:])
        wA = pool.tile([P, NDC, ADW], BF16)
        dma(out=wA[:], in_=t["w_ad_mine"][:])
        bA = pool.tile([1, ADW], F32)
        dma(out=bA[:], in_=t["b_ad_mine"][:])

        sc_f = pool.tile([P, NDC], F32)
        act(sc_f[:], cond_sb[:], AF.Silu)
        sc_bf = pool.tile([P, NDC], BF16)
        v.tensor_copy(sc_bf[:], sc_f[:])

        row_sb = pool.tile([1, ADW], F32)
        for li in range(5):
            ps = psA.tile([1, P], F32)
            for dc in range(NDC):
                mm(
                    ps[:],
                    sc_bf[:, dc : dc + 1],
                    wA[:, dc, li * P : (li + 1) * P],
                    start=(dc == 0),
                    stop=(dc == NDC - 1),
                )
            v.tensor_add(
                row_sb[:, li * P : (li + 1) * P], ps[:], bA[:, li * P : (li + 1) * P]
            )
        dma(out=t["ad_in"][:], in_=row_sb[:])
        nc.gpsimd.collective_compute(
            kind="AllGather",
            op=OP.bypass,
            replica_groups=rg,
            ins=[t["ad_in"][:]],
            outs=[t["ad_out"][:]],
        )
        # columns [128, 36]: chunks 0-19 live on cores 0-3 (5 each),
        # chunks 20-35 on cores 4-7 (4 each)
        ad_r = t["ad_out"].rearrange("c (l p) -> p c l", p=P)
        for cc in range(4):
            dma(out=colmat[:, 5 * cc : 5 * cc + 5], in_=ad_r[:, cc, :])
        for cc in range(4, 8):
            dma(
                out=colmat[:, 20 + 4 * (cc - 4) : 24 + 4 * (cc - 4)],
                in_=ad_r[:, cc, 0:4],
            )
        v.tensor_scalar_add(sp1[:], colmat[:, 6:12], 1.0)
        v.tensor_scalar_add(sp2[:], colmat[:, 24:30], 1.0)
        # gate rows: chunks 12-17 -> (2,2..4)+(3,0..2); 30-35 -> (6,2..3)+(7,0..3)
        g_row1 = pool.tile([1, D], F32)
        dma(out=g_row1[:, 0:384], in_=t["ad_out"][2:3, 2 * P : 5 * P])
        dma(out=g_row1[:, 384:768], in_=t["ad_out"][3:4, 0 : 3 * P])
        g_row2 = pool.tile([1, D], F32)
        dma(out=g_row2[:, 0:256], in_=t["ad_out"][6:7, 2 * P : 4 * P])
        dma(out=g_row2[:, 256:768], in_=t["ad_out"][7:8, 0 : 4 * P])
        nc.gpsimd.partition_broadcast(g1_b[:], g_row1[:])
        nc.gpsimd.partition_broadcast(g2_b[:], g_row2[:])

    # ------- phase B: local LN1 + modulation + Q/K/V + K/V gather ---------
    x_loc = big.tile([P, NQC, D], F32)
    dma(out=x_loc[:], in_=t["x_loc"].rearrange("(n p) d -> p n d", p=P))

    with ExitStack() as phB:
        pool = phB.enter_context(tc.tile_pool(name="phB", bufs=1))
        spool = phB.enter_context(tc.tile_pool(name="spoolB", bufs=3))
        nxpool = phB.enter_context(tc.tile_pool(name="nxB", bufs=2))
        psV = phB.enter_context(tc.tile_pool(name="psV", bufs=2, space="PSUM"))
        psK = phB.enter_context(tc.tile_pool(name="psK", bufs=2, space="PSUM"))
        psT = phB.enter_context(tc.tile_pool(name="psT", bufs=2, space="PSUM"))

        wqkv = pool.tile([P, NDC, 3 * D], BF16)
        dma(out=wqkv[:], in_=t["w_qkv"].rearrange("(c p) m -> p c m", p=P))
        bq_col = pool.tile([P, NHP], F32)
        dma(out=bq_col[:], in_=t["bq_col"][:])
        bk_col = pool.tile([P, NHP], F32)
        dma(out=bk_col[:], in_=t["bk_col"][:])
        bv_b = pool.tile([P, D], F32)
        dma(out=bv_b[:], in_=t["bv_b"][:])

        xn1T = pool.tile([P, NDC, LQ], BF16)
        for i in range(NQC):
            xt = x_loc[:, i, :]
            mv, rstd = _ln_rstd(nc, spool, eps_t, xt)
            nx = nxpool.tile([P, D], BF16)
            v.tensor_scalar(
                nx[:], xt, mv[:, 0:1], rstd[:], op0=OP.subtract, op1=OP.mult
            )
            for dc in range(NDC):
                pt = psT.tile([P, P], BF16)
                nc.tensor.transpose(pt[:], nx[:, dc * P : (dc + 1) * P], identity[:])
                act(
                    xn1T[:, dc, i * P : (i + 1) * P],
                    pt[:],
                    AF.Identity,
                    bias=colmat[:, dc : dc + 1],
                    scale=sp1[:, dc : dc + 1],
                )

        # V (natural layout, keys on partitions, ones column interleaved)
        v_loc = pool.tile([P, NQC, 12 * VW], BF16)
        v.memset(v_loc.rearrange("p k (h e) -> p k h e", e=VW)[:, :, :, HD:VW], 1.0)
        v4 = v_loc.rearrange("p k (h e) -> p k h e", e=VW)
        bv3 = bv_b.rearrange("p (h e) -> p h e", e=HD)
        for kc in range(NQC):
            ps_v = psV.tile([P, D], F32)
            lhs = xn1T[:, :, kc * P : (kc + 1) * P]
            for dc in range(NDC):
                mm(ps_v[:, 0:512], lhs[:, dc, :], wqkv[:, dc, 2 * D : 2 * D + 512],
                   start=(dc == 0), stop=(dc == NDC - 1))
                mm(ps_v[:, 512:768], lhs[:, dc, :], wqkv[:, dc, 2 * D + 512 : 3 * D],
                   start=(dc == 0), stop=(dc == NDC - 1))
            v.tensor_tensor(
                v4[:, kc, :, 0:HD],
                ps_v.rearrange("p (h e) -> p h e", e=HD),
                bv3[:],
                op=OP.add,
            )

        # K^T then Q^T per head pair
        kT_loc = pool.tile([P, NHP, LQ], BF16)
        for hp in range(NHP):
            ps_k = psK.tile([P, LQ], F32)
            for dc in range(NDC):
                mm(
                    ps_k[:],
                    wqkv[:, dc, D + hp * P : D + (hp + 1) * P],
                    xn1T[:, dc, :],
                    start=(dc == 0),
                    stop=(dc == NDC - 1),
                )
            v.tensor_scalar_add(kT_loc[:, hp, :], ps_k[:], bk_col[:, hp : hp + 1])
        # kick each group's collective as soon as its K and V slices exist
        for g in range(NG):
            for j in range(2):
                hp = 2 * g + j
                dma(
                    out=t["kv_in"][g][:, j * LQ : (j + 1) * LQ],
                    in_=kT_loc[:, hp, :],
                )
            for kc in range(NQC):
                dma(
                    out=t["kv_in"][g][:, 2 * LQ + kc * GW : 2 * LQ + (kc + 1) * GW],
                    in_=v_loc[:, kc, g * GW : (g + 1) * GW],
                )
            nc.gpsimd.collective_compute(
                kind="AllGather",
                op=OP.bypass,
                replica_groups=rg,
                ins=[t["kv_in"][g][:]],
                outs=[t["kv_out"][g][:]],
            )
        for hp in range(NHP):
            ps_q = psK.tile([P, LQ], F32, tag="ps_k")
            for dc in range(NDC):
                mm(
                    ps_q[:],
                    wqkv[:, dc, hp * P : (hp + 1) * P],
                    xn1T[:, dc, :],
                    start=(dc == 0),
                    stop=(dc == NDC - 1),
                )
            v.tensor_scalar_add(qT_all[:, hp, :], ps_q[:], bq_col[:, hp : hp + 1])

    # ---------------- phase C: attention over gathered K/V ----------------
    # preload the out-proj and FFN1 weights while attention runs
    wpool = ctx.enter_context(tc.tile_pool(name="wDF", bufs=1))
    wao_sb = wpool.tile([P, NDC, D], BF16)
    dma(out=wao_sb[:], in_=t["w_attn_out"].rearrange("(c p) m -> p c m", p=P))
    ba_sb = wpool.tile([P, D], F32)
    dma(out=ba_sb[:], in_=t["b_attn_b"][:])
    wf1_sb = wpool.tile([P, NDC, DM], BF16)
    dma(out=wf1_sb[:], in_=t["w_ffn1"].rearrange("(c p) m -> p c m", p=P))
    bf1_col = wpool.tile([P, NMC], F32)
    dma(out=bf1_col[:], in_=t["b_ffn1_col"][:])

    with ExitStack() as phC:
        kpool = phC.enter_context(tc.tile_pool(name="kp", bufs=2))
        vpool = phC.enter_context(tc.tile_pool(name="vp", bufs=2))
        ppool = phC.enter_context(tc.tile_pool(name="pp", bufs=3))
        tiny = phC.enter_context(tc.tile_pool(name="tiny", bufs=2))
        rzb_pool = phC.enter_context(tc.tile_pool(name="rzb", bufs=2))
        psS = phC.enter_context(tc.tile_pool(name="psS", bufs=2, space="PSUM"))
        psO = phC.enter_context(tc.tile_pool(name="psO", bufs=3, space="PSUM"))

        for g in range(NG):
            kv_r = t["kv_out"][g].rearrange("(c p) m -> p c m", p=P)
            kT_g = kpool.tile([P, 2, L], BF16, tag="k")
            for j in range(2):
                dma(
                    out=kT_g.rearrange("p h (c s) -> p h c s", s=LQ)[:, j, :, :],
                    in_=kv_r[:, :, j * LQ : (j + 1) * LQ],
                )
            v_g = vpool.tile([P, L // P, GW], BF16, tag="v")
            dma(
                out=v_g.rearrange("p (c k) e -> p c k e", c=NCORES),
                in_=kv_r[:, :, 2 * LQ : CCW].rearrange(
                    "p c (k e) -> p c k e", e=GW
                ),
            )
            kT4 = kT_g.rearrange("p h (k s) -> p h k s", s=P)
            for hl in range(4):
                h = 4 * g + hl
                hp, off = h // 2, (h % 2) * HD
                ps_o = psO.tile([VW, LQ], F32)
                for kc2 in range(16):
                    ps_s = psS.tile([P, 1024], F32)
                    for j in range(2):
                        kc = 2 * kc2 + j
                        mm(
                            ps_s[:, j * LQ : (j + 1) * LQ],
                            kT4[off : off + HD, hl // 2, kc, :],
                            qT_all[off : off + HD, hp, :],
                            start=True,
                            stop=True,
                        )
                    ptile = ppool.tile([P, 1024], BF16)
                    act(ptile[:], ps_s[:], AF.Exp, scale=0.125)
                    for j in range(2):
                        kc = 2 * kc2 + j
                        mm(
                            ps_o[:],
                            v_g[:, kc, hl * VW : (hl + 1) * VW],
                            ptile[:, j * LQ : (j + 1) * LQ],
                            start=(kc == 0),
                            stop=(kc == L // P - 1),
                        )
                zrow = tiny.tile([1, LQ], F32, tag="zrow")
                v.tensor_copy(zrow[:], ps_o[HD : HD + 1, :])
                rz = tiny.tile([1, LQ], F32, tag="rz")
                v.reciprocal_approx_fast(rz[:], zrow[:])
                rz_b = rzb_pool.tile([P, LQ], F32)
                nc.gpsimd.partition_broadcast(rz_b[:], rz[:])
                v.tensor_tensor(
                    catT_all[off : off + HD, hp, :],
                    ps_o[0:HD, :],
                    rz_b[0:HD, :],
                    op=OP.mult,
                )

    # FFN2 weights land while the out-projection/LN2/FFN1 run (kv space freed)
    w2pool = ctx.enter_context(tc.tile_pool(name="wF2", bufs=1))
    wf2_sb = w2pool.tile([P, NMC, D], BF16)
    dma(out=wf2_sb[:], in_=t["w_ffn2"].rearrange("(c p) m -> p c m", p=P))
    bf2_b = w2pool.tile([P, D], F32)
    dma(out=bf2_b[:], in_=t["b_ffn2_b"][:])

    # -------- phase D: out-projection, gate, residual, LN2 ----------------
    with ExitStack() as phD:
        pool = phD.enter_context(tc.tile_pool(name="phD", bufs=2))
        spool = phD.enter_context(tc.tile_pool(name="spoolD", bufs=3))
        nxpool = phD.enter_context(tc.tile_pool(name="nxD", bufs=2))
        psD1 = phD.enter_context(tc.tile_pool(name="psD1", bufs=2, space="PSUM"))
        psD2 = phD.enter_context(tc.tile_pool(name="psD2", bufs=2, space="PSUM"))
        psT = phD.enter_context(tc.tile_pool(name="psTD", bufs=2, space="PSUM"))

        for qc in range(NQC):
            ps1 = psD1.tile([P, 512], F32)
            ps2 = psD2.tile([P, 256], F32)
            for cc in range(NDC):
                lhs = catT_all[:, cc, qc * P : (qc + 1) * P]
                mm(ps1[:], lhs, wao_sb[:, cc, 0:512],
                   start=(cc == 0), stop=(cc == NDC - 1))
                mm(ps2[:], lhs, wao_sb[:, cc, 512:768],
                   start=(cc == 0), stop=(cc == NDC - 1))
            yb = pool.tile([P, D], F32)
            v.tensor_add(yb[:, 0:512], ps1[:], ba_sb[:, 0:512])
            v.tensor_add(yb[:, 512:768], ps2[:], ba_sb[:, 512:768])
            yg = pool.tile([P, D], F32)
            v.tensor_tensor(yg[:], yb[:], g1_b[:], op=OP.mult)
            v.tensor_add(x2_loc[qc][:], yg[:], x_loc[:, qc, :])
            xq = x2_loc[qc][:]
            mv, rstd = _ln_rstd(nc, spool, eps_t, xq)
            nx = nxpool.tile([P, D], BF16)
            v.tensor_scalar(
                nx[:], xq, mv[:, 0:1], rstd[:], op0=OP.subtract, op1=OP.mult
            )
            for dc in range(NDC):
                pt = psT.tile([P, P], BF16)
                nc.tensor.transpose(pt[:], nx[:, dc * P : (dc + 1) * P], identity[:])
                act(
                    xn2T[:, dc, qc * P : (qc + 1) * P],
                    pt[:],
                    AF.Identity,
                    bias=colmat[:, 18 + dc : 19 + dc],
                    scale=sp2[:, dc : dc + 1],
                )
    # ---------------- phase F: FFN + gate + residual -> output -------------
    with ExitStack() as phF:
        hpool = phF.enter_context(tc.tile_pool(name="hT", bufs=1))
        pool = phF.enter_context(tc.tile_pool(name="phF", bufs=2))
        psF1 = phF.enter_context(tc.tile_pool(name="psF1", bufs=3, space="PSUM"))
        psF2 = phF.enter_context(tc.tile_pool(name="psF2", bufs=2, space="PSUM"))

        hT_all = hpool.tile([P, NMC, LQ], BF16)
        for mc in range(NMC):
            ps_h = psF1.tile([P, LQ], F32, tag="mm512")
            for dc in range(NDC):
                mm(
                    ps_h[:],
                    wf1_sb[:, dc, mc * P : (mc + 1) * P],
                    xn2T[:, dc, :],
                    start=(dc == 0),
                    stop=(dc == NDC - 1),
                )
            act(hT_all[:, mc, :], ps_h[:], AF.Gelu, bias=bf1_col[:, mc : mc + 1])

        out_r = t["out"].rearrange("(n p) d -> n p d", p=P)
        for qc in range(NQC):
            ps1 = psF1.tile([P, 512], F32, tag="mm512")
            ps2 = psF2.tile([P, 256], F32)
            for mc in range(NMC):
                lhs = hT_all[:, mc, qc * P : (qc + 1) * P]
                mm(ps1[:], lhs, wf2_sb[:, mc, 0:512],
                   start=(mc == 0), stop=(mc == NMC - 1))
                mm(ps2[:], lhs, wf2_sb[:, mc, 512:768],
                   start=(mc == 0), stop=(mc == NMC - 1))
            y2 = pool.tile([P, D], F32)
            v.tensor_add(y2[:, 0:512], ps1[:], bf2_b[:, 0:512])
            v.tensor_add(y2[:, 512:768], ps2[:], bf2_b[:, 512:768])
            yg = pool.tile([P, D], F32)
            v.tensor_tensor(yg[:], y2[:], g2_b[:], op=OP.mult)
            ot = pool.tile([P, D], F32)
            v.tensor_add(ot[:], yg[:], x2_loc[qc][:])
            dma(out=out_r[qc], in_=ot[:])


def build_nc():
    nc = bacc.Bacc(
        None, target_bir_lowering=False, debug=False, num_devices=NCORES
    )
    t = _declare_params(nc)
    with tile.TileContext(nc) as tc:
        with ExitStack() as ctx:
            _build_body(nc, tc, ctx, t)
    nc.compile()
    return nc


_cache = {}


def _prep_in_maps(inputs):
    bf = lambda a: np.ascontiguousarray(np.asarray(a, np.float32)).astype(
        ml_dtypes.bfloat16
    )
    f32 = lambda a: np.ascontiguousarray(np.asarray(a, np.float32))
    x = f32(inputs["x"]).reshape(L, D)
    cond = f32(inputs["cond"]).reshape(D)
    b_qkv = f32(inputs["b_qkv"]).reshape(3 * D)
    # concatenated adaln weights/biases in chunk space (36 chunks of 128)
    wcat = np.concatenate(
        [f32(inputs["w_adaln1"]), f32(inputs["w_adaln2"])], axis=1
    )  # [D, 4608]
    bcat = np.concatenate(
        [f32(inputs["b_adaln1"]).reshape(-1), f32(inputs["b_adaln2"]).reshape(-1)]
    )  # [4608]
    common = {
        "cond_t": np.ascontiguousarray(cond.reshape(NDC, P).T),
        "w_qkv": bf(inputs["w_qkv"]),
        "bq_col": np.ascontiguousarray(b_qkv[:D].reshape(NHP, P).T),
        "bk_col": np.ascontiguousarray(b_qkv[D : 2 * D].reshape(NHP, P).T),
        "bv_b": np.ascontiguousarray(np.broadcast_to(b_qkv[2 * D :], (P, D))),
        "w_attn_out": bf(inputs["w_attn_out"]),
        "b_attn_b": np.ascontiguousarray(
            np.broadcast_to(f32(inputs["b_attn_out"]).reshape(D), (P, D))
        ),
        "w_ffn1": bf(inputs["w_ffn1"]),
        "b_ffn1_col": np.ascontiguousarray(
            f32(inputs["b_ffn1"]).reshape(NMC, P).T
        ),
        "w_ffn2": bf(inputs["w_ffn2"]),
        "b_ffn2_b": np.ascontiguousarray(
            np.broadcast_to(f32(inputs["b_ffn2"]).reshape(D), (P, D))
        ),
    }
    in_maps = []
    for c in range(NCORES):
        m = dict(common)
        m["x_loc"] = np.ascontiguousarray(x[c * LQ : (c + 1) * LQ])
        # this core's adaln chunks (pad to 5)
        chunks = (
            range(5 * c, 5 * c + 5) if c < 4 else range(20 + 4 * (c - 4), 24 + 4 * (c - 4))
        )
        wm = np.zeros((D, ADW), np.float32)
        bm = np.zeros((1, ADW), np.float32)
        for li, k in enumerate(chunks):
            wm[:, li * P : (li + 1) * P] = wcat[:, k * P : (k + 1) * P]
            bm[0, li * P : (li + 1) * P] = bcat[k * P : (k + 1) * P]
        m["w_ad_mine"] = np.ascontiguousarray(
            wm.reshape(NDC, P, ADW).transpose(1, 0, 2)
        ).astype(ml_dtypes.bfloat16)
        m["b_ad_mine"] = bm
        in_maps.append(m)
    return in_maps


def kernel(**inputs):
    if "nc" not in _cache:
        _cache["nc"] = build_nc()
    nc = _cache["nc"]
    in_maps = _prep_in_maps(inputs)
    res = run_bass_kernel_spmd(nc, in_maps, list(range(NCORES)))
    out = np.concatenate([res.results[c]["out"] for c in range(NCORES)], axis=0)
    return out.reshape(1, L, D).astype(np.float32)


if __name__ == "__main__":
    rng = np.random.default_rng(0)
    fake = {
        "x": rng.standard_normal((1, L, D), dtype=np.float32),
        "cond": rng.standard_normal((1, D), dtype=np.float32),
        "w_adaln1": rng.standard_normal((D, 3 * D), dtype=np.float32) * 0.02,
        "b_adaln1": np.zeros(3 * D, np.float32),
        "w_qkv": rng.standard_normal((D, 3 * D), dtype=np.float32) * D**-0.5,
        "b_qkv": np.zeros(3 * D, np.float32),
        "w_attn_out": rng.standard_normal((D, D), dtype=np.float32) * D**-0.5,
        "b_attn_out": np.zeros(D, np.float32),
        "w_adaln2": rng.standard_normal((D, 3 * D), dtype=np.float32) * 0.02,
        "b_adaln2": np.zeros(3 * D, np.float32),
        "w_ffn1": rng.standard_normal((D, DM), dtype=np.float32) * D**-0.5,
        "b_ffn1": np.zeros(DM, np.float32),
        "w_ffn2": rng.standard_normal((DM, D), dtype=np.float32) * DM**-0.5,
        "b_ffn2": np.zeros(D, np.float32),
    }
    out = kernel(**fake)
    print("out", out.shape, out.dtype, np.abs(out).max())


# revision 14
# speedup vs baseline: 1.3921x; 1.3921x over previous
"""DiT block kernel for 8 Trainium2 NeuronCores (Bass/Tile).

Sharding: sequence dim L=4096 split 8 ways (512 rows per core). Each core
computes K/V only for its OWN 512 rows, then an HBM AllGather distributes
the full K/V to every core (3 pipelined group collectives, 4 heads each).
The tiny AdaLN cond-projection GEMV is also sharded 8 ways and gathered.
Attention, out-projection and the FFN run on the local 512 rows only.
Weights are bf16; accumulation is fp32 in PSUM; layernorm statistics and
residuals are fp32. Softmax uses an augmented-V ones column for the
denominator (no max subtraction; logits are ~N(0,1) after 1/8 scaling).
"""

import sys

sys.path.insert(0, "/opt/trn_rl_repo")

from contextlib import ExitStack

import numpy as np
import ml_dtypes

import concourse.bass as bass
import concourse.bacc as bacc
import concourse.tile as tile
import concourse.mybir as mybir
from concourse.bass_utils import run_bass_kernel_spmd
from concourse.masks import make_identity

F32 = mybir.dt.float32
BF16 = mybir.dt.bfloat16
AF = mybir.ActivationFunctionType
OP = mybir.AluOpType

L, D, H, HD, DM = 4096, 768, 12, 64, 3072
NCORES = 8
LQ = L // NCORES  # 512 local rows
P = 128
EPS = 1e-5
NDC = D // P  # 6 chunks of the model dim
NHP = H // 2  # 6 head pairs
NMC = DM // P  # 24 chunks of the FFN hidden dim
NQC = LQ // P  # 4 local row chunks
NG = 3  # head groups for the K/V collective (4 heads each)
VW = HD + 1  # V channels per head incl ones column
GW = 4 * VW  # V channels per group (260)
CCW = 2 * LQ + 4 * GW  # collective width per group: kT(1024) + v(1040)

# AdaLN chunk ownership: 36 chunks of 128 over the concatenated
# [adaln1 | adaln2] output (2*2304 = 4608). Cores 0-3 own 5 chunks,
# cores 4-7 own 4 (+1 pad so the gather is uniform at 640 floats).
ADW = 640


def _ad_owner(k):
    if k < 20:
        return k // 5, k % 5
    return 4 + (k - 20) // 4, (k - 20) % 4


def _declare_params(nc):
    dp = nc.declare_dram_parameter
    t = {}
    t["x_loc"] = dp("x_loc", [LQ, D], F32, isOutput=False)
    t["cond_t"] = dp("cond_t", [P, NDC], F32, isOutput=False)
    t["w_adaln1"] = dp("w_adaln1", [D, 3 * D], BF16, isOutput=False)
    t["w_adaln2"] = dp("w_adaln2", [D, 3 * D], BF16, isOutput=False)
    t["b_adaln1_col"] = dp("b_adaln1_col", [P, 12], F32, isOutput=False)
    t["b_adaln2_col"] = dp("b_adaln2_col", [P, 12], F32, isOutput=False)
    t["b_adaln1_gate"] = dp("b_adaln1_gate", [1, D], F32, isOutput=False)
    t["b_adaln2_gate"] = dp("b_adaln2_gate", [1, D], F32, isOutput=False)
    t["w_qkv"] = dp("w_qkv", [D, 3 * D], BF16, isOutput=False)
    t["bq_col"] = dp("bq_col", [P, NHP], F32, isOutput=False)
    t["bk_col"] = dp("bk_col", [P, NHP], F32, isOutput=False)
    t["bv_b"] = dp("bv_b", [P, D], F32, isOutput=False)
    t["w_attn_out"] = dp("w_attn_out", [D, D], BF16, isOutput=False)
    t["b_attn_b"] = dp("b_attn_b", [P, D], F32, isOutput=False)
    t["w_ffn1"] = dp("w_ffn1", [D, DM], BF16, isOutput=False)
    t["b_ffn1_col"] = dp("b_ffn1_col", [P, NMC], F32, isOutput=False)
    t["w_ffn2"] = dp("w_ffn2", [DM, D], BF16, isOutput=False)
    t["b_ffn2_b"] = dp("b_ffn2_b", [P, D], F32, isOutput=False)
    t["out"] = dp("out", [LQ, D], F32, isOutput=True)
    # DRAM scratch for the collectives
    t["dum_in"] = nc.dram_tensor("dum_in", [1, 2], F32)
    t["dum_out"] = nc.dram_tensor("dum_out", [NCORES, 2], F32, addr_space="Shared")
    t["kv_in"] = [nc.dram_tensor(f"kv_in{g}", [P, CCW], BF16) for g in range(NG)]
    t["kv_out"] = [
        nc.dram_tensor(f"kv_out{g}", [NCORES * P, CCW], BF16, addr_space="Shared")
        for g in range(NG)
    ]
    return t


def _ln_rstd(nc, spool, eps_t, xt, halves=2):
    """LayerNorm stats for a [P, D] f32 tile -> (mean [P,1], rstd [P,1]).

    rstd = exp(-0.5*ln(var+eps)) keeps the Act engine on the ln/exp table
    (no sqrt-table switch next to the softmax exp)."""
    v = nc.vector
    act = nc.scalar.activation
    stats = spool.tile([P, halves, 6], F32)
    w = D // halves
    for gki in range(halves):
        v.bn_stats(stats[:, gki, :], xt[:, gki * w : (gki + 1) * w])
    mv = spool.tile([P, 2], F32)
    v.bn_aggr(mv[:], stats[:])
    lnv = spool.tile([P, 1], F32)
    act(lnv[:], mv[:, 1:2], AF.Ln, bias=eps_t[:, 0:1])
    rstd = spool.tile([P, 1], F32)
    act(rstd[:], lnv[:], AF.Exp, scale=-0.5)
    return mv, rstd


def _build_body(nc, tc, ctx, t):
    mm = nc.tensor.matmul
    dma = nc.sync.dma_start
    v = nc.vector
    act = nc.scalar.activation
    rg = [list(range(NCORES))]

    # dependency-free warmup collective: absorbs the one-time NRT channel
    # setup latency while phase A/B compute runs
    nc.gpsimd.collective_compute(
        kind="AllGather",
        op=OP.bypass,
        replica_groups=rg,
        ins=[t["dum_in"][:]],
        outs=[t["dum_out"][:]],
    )

    const = ctx.enter_context(tc.tile_pool(name="const", bufs=1))
    identity = const.tile([P, P], BF16)
    make_identity(nc, identity)
    eps_t = const.tile([P, 1], F32)
    v.memset(eps_t, EPS)

    # Long-lived tiles
    big = ctx.enter_context(tc.tile_pool(name="big", bufs=1))
    sh1 = big.tile([P, NDC], F32)  # shift1, column layout
    sh2 = big.tile([P, NDC], F32)
    sp1 = big.tile([P, NDC], F32)  # 1 + scale1
    sp2 = big.tile([P, NDC], F32)
    g1_b = big.tile([P, D], F32)  # gate1 broadcast over partitions
    g2_b = big.tile([P, D], F32)
    qT_all = big.tile([P, NHP, LQ], BF16)
    catT_all = big.tile([P, NDC, LQ], BF16)  # attention output, transposed
    xn2T = big.tile([P, NDC, LQ], BF16)
    x2_loc = [big.tile([P, D], F32, name=f"x2_{q}") for q in range(NQC)]

    # -------- phase A: replicated AdaLN cond projection (no collective) ----
    with ExitStack() as phA:
        pool = phA.enter_context(tc.tile_pool(name="phA", bufs=1))
        psA1 = phA.enter_context(tc.tile_pool(name="psA1", bufs=2, space="PSUM"))
        psA2 = phA.enter_context(tc.tile_pool(name="psA2", bufs=2, space="PSUM"))

        cond_sb = pool.tile([P, NDC], F32)
        dma(out=cond_sb[:], in_=t["cond_t"][:])
        sc_f = pool.tile([P, NDC], F32)
        act(sc_f[:], cond_sb[:], AF.Silu)
        sc_bf = pool.tile([P, NDC], BF16)
        v.tensor_copy(sc_bf[:], sc_f[:])

        wa1 = pool.tile([P, NDC, 3 * D], BF16)
        dma(out=wa1[:], in_=t["w_adaln1"].rearrange("(c p) m -> p c m", p=P))
        wa2 = pool.tile([P, NDC, 3 * D], BF16)
        dma(out=wa2[:], in_=t["w_adaln2"].rearrange("(c p) m -> p c m", p=P))
        b1c = pool.tile([P, 12], F32)
        dma(out=b1c[:], in_=t["b_adaln1_col"][:])
        b2c = pool.tile([P, 12], F32)
        dma(out=b2c[:], in_=t["b_adaln2_col"][:])
        b1g = pool.tile([1, D], F32)
        dma(out=b1g[:], in_=t["b_adaln1_gate"][:])
        b2g = pool.tile([1, D], F32)
        dma(out=b2g[:], in_=t["b_adaln2_gate"][:])

        for r, (wa, bc, bg, sh, sp, g_b) in enumerate(
            [
                (wa1, b1c, b1g, sh1, sp1, g1_b),
                (wa2, b2c, b2g, sh2, sp2, g2_b),
            ]
        ):
            acol = pool.tile([P, 12], F32, name=f"acol{r}")
            for m in range(12):
                ps = psA1.tile([P, 1], F32)
                for dc in range(NDC):
                    mm(
                        ps[:],
                        wa[:, dc, m * P : (m + 1) * P],
                        sc_bf[:, dc : dc + 1],
                        start=(dc == 0),
                        stop=(dc == NDC - 1),
                    )
                v.tensor_add(acol[:, m : m + 1], ps[:], bc[:, m : m + 1])
            v.tensor_copy(sh[:], acol[:, 0:6])
            v.tensor_scalar_add(sp[:], acol[:, 6:12], 1.0)
            g_row = pool.tile([1, D], F32, name=f"grow{r}")
            for n0, n1 in [(0, 512), (512, 768)]:
                ps2 = psA2.tile([1, n1 - n0], F32, tag="psg")
                for dc in range(NDC):
                    mm(
                        ps2[:],
                        sc_bf[:, dc : dc + 1],
                        wa[:, dc, 2 * D + n0 : 2 * D + n1],
                        start=(dc == 0),
                        stop=(dc == NDC - 1),
                    )
                v.tensor_add(g_row[:, n0:n1], ps2[:], bg[:, n0:n1])
            nc.gpsimd.partition_broadcast(g_b[:], g_row[:])

    # ------- phase B: local LN1 + modulation + Q/K/V + K/V gather ---------
    x_loc = big.tile([P, NQC, D], F32)
    dma(out=x_loc[:], in_=t["x_loc"].rearrange("(n p) d -> p n d", p=P))

    with ExitStack() as phB:
        pool = phB.enter_context(tc.tile_pool(name="phB", bufs=1))
        spool = phB.enter_context(tc.tile_pool(name="spoolB", bufs=3))
        nxpool = phB.enter_context(tc.tile_pool(name="nxB", bufs=2))
        psV = phB.enter_context(tc.tile_pool(name="psV", bufs=2, space="PSUM"))
        psK = phB.enter_context(tc.tile_pool(name="psK", bufs=2, space="PSUM"))
        psT = phB.enter_context(tc.tile_pool(name="psT", bufs=2, space="PSUM"))

        wqkv = pool.tile([P, NDC, 3 * D], BF16)
        dma(out=wqkv[:], in_=t["w_qkv"].rearrange("(c p) m -> p c m", p=P))
        bq_col = pool.tile([P, NHP], F32)
        dma(out=bq_col[:], in_=t["bq_col"][:])
        bk_col = pool.tile([P, NHP], F32)
        dma(out=bk_col[:], in_=t["bk_col"][:])
        bv_b = pool.tile([P, D], F32)
        dma(out=bv_b[:], in_=t["bv_b"][:])

        xn1T = pool.tile([P, NDC, LQ], BF16)
        for i in range(NQC):
            xt = x_loc[:, i, :]
            mv, rstd = _ln_rstd(nc, spool, eps_t, xt)
            nx = nxpool.tile([P, D], BF16)
            v.tensor_scalar(
                nx[:], xt, mv[:, 0:1], rstd[:], op0=OP.subtract, op1=OP.mult
            )
            for dc in range(NDC):
                pt = psT.tile([P, P], BF16)
                nc.tensor.transpose(pt[:], nx[:, dc * P : (dc + 1) * P], identity[:])
                act(
                    xn1T[:, dc, i * P : (i + 1) * P],
                    pt[:],
                    AF.Identity,
                    bias=sh1[:, dc : dc + 1],
                    scale=sp1[:, dc : dc + 1],
                )

        # V (natural layout, keys on partitions, ones column interleaved)
        v_loc = pool.tile([P, NQC, 12 * VW], BF16)
        v.memset(v_loc.rearrange("p k (h e) -> p k h e", e=VW)[:, :, :, HD:VW], 1.0)
        v4 = v_loc.rearrange("p k (h e) -> p k h e", e=VW)
        bv3 = bv_b.rearrange("p (h e) -> p h e", e=HD)
        for kc in range(NQC):
            ps_v = psV.tile([P, D], F32)
            lhs = xn1T[:, :, kc * P : (kc + 1) * P]
            for dc in range(NDC):
                mm(ps_v[:, 0:512], lhs[:, dc, :], wqkv[:, dc, 2 * D : 2 * D + 512],
                   start=(dc == 0), stop=(dc == NDC - 1))
                mm(ps_v[:, 512:768], lhs[:, dc, :], wqkv[:, dc, 2 * D + 512 : 3 * D],
                   start=(dc == 0), stop=(dc == NDC - 1))
            v.tensor_tensor(
                v4[:, kc, :, 0:HD],
                ps_v.rearrange("p (h e) -> p h e", e=HD),
                bv3[:],
                op=OP.add,
            )

        # K^T per head pair; kick each group's collective as soon as its
        # K head-pairs and V slices exist
        kT_loc = pool.tile([P, NHP, LQ], BF16)
        for g in range(NG):
            for j in range(2):
                hp = 2 * g + j
                ps_k = psK.tile([P, LQ], F32)
                for dc in range(NDC):
                    mm(
                        ps_k[:],
                        wqkv[:, dc, D + hp * P : D + (hp + 1) * P],
                        xn1T[:, dc, :],
                        start=(dc == 0),
                        stop=(dc == NDC - 1),
                    )
                v.tensor_scalar_add(
                    kT_loc[:, hp, :], ps_k[:], bk_col[:, hp : hp + 1]
                )
                dma(
                    out=t["kv_in"][g][:, j * LQ : (j + 1) * LQ],
                    in_=kT_loc[:, hp, :],
                )
            for kc in range(NQC):
                dma(
                    out=t["kv_in"][g][:, 2 * LQ + kc * GW : 2 * LQ + (kc + 1) * GW],
                    in_=v_loc[:, kc, g * GW : (g + 1) * GW],
                )
            nc.gpsimd.collective_compute(
                kind="AllGather",
                op=OP.bypass,
                replica_groups=rg,
                ins=[t["kv_in"][g][:]],
                outs=[t["kv_out"][g][:]],
            )
        for hp in range(NHP):
            ps_q = psK.tile([P, LQ], F32, tag="ps_k")
            for dc in range(NDC):
                mm(
                    ps_q[:],
                    wqkv[:, dc, hp * P : (hp + 1) * P],
                    xn1T[:, dc, :],
                    start=(dc == 0),
                    stop=(dc == NDC - 1),
                )
            v.tensor_scalar_add(qT_all[:, hp, :], ps_q[:], bq_col[:, hp : hp + 1])

    # ---------------- phase C: attention over gathered K/V ----------------
    # preload the out-proj and FFN1 weights while attention runs
    wpool = ctx.enter_context(tc.tile_pool(name="wDF", bufs=1))
    wao_sb = wpool.tile([P, NDC, D], BF16)
    dma(out=wao_sb[:], in_=t["w_attn_out"].rearrange("(c p) m -> p c m", p=P))
    ba_sb = wpool.tile([P, D], F32)
    dma(out=ba_sb[:], in_=t["b_attn_b"][:])
    wf1_sb = wpool.tile([P, NDC, DM], BF16)
    dma(out=wf1_sb[:], in_=t["w_ffn1"].rearrange("(c p) m -> p c m", p=P))
    bf1_col = wpool.tile([P, NMC], F32)
    dma(out=bf1_col[:], in_=t["b_ffn1_col"][:])

    with ExitStack() as phC:
        kpool = phC.enter_context(tc.tile_pool(name="kp", bufs=2))
        vpool = phC.enter_context(tc.tile_pool(name="vp", bufs=2))
        ppool = phC.enter_context(tc.tile_pool(name="pp", bufs=4))
        tiny = phC.enter_context(tc.tile_pool(name="tiny", bufs=2))
        rzb_pool = phC.enter_context(tc.tile_pool(name="rzb", bufs=2))
        psS = phC.enter_context(tc.tile_pool(name="psS", bufs=3, space="PSUM"))
        psO = phC.enter_context(tc.tile_pool(name="psO", bufs=2, space="PSUM"))

        for g in range(NG):
            kv_r = t["kv_out"][g].rearrange("(c p) m -> p c m", p=P)
            kT_g = kpool.tile([P, 2, L], BF16, tag="k")
            for j in range(2):
                dma(
                    out=kT_g.rearrange("p h (c s) -> p h c s", s=LQ)[:, j, :, :],
                    in_=kv_r[:, :, j * LQ : (j + 1) * LQ],
                )
            v_g = vpool.tile([P, L // P, GW], BF16, tag="v")
            dma(
                out=v_g.rearrange("p (c k) e -> p c k e", c=NCORES),
                in_=kv_r[:, :, 2 * LQ : CCW].rearrange(
                    "p c (k e) -> p c k e", e=GW
                ),
            )
            kT4 = kT_g.rearrange("p h (k s) -> p h k s", s=P)
            for hl in range(4):
                h = 4 * g + hl
                hp, off = h // 2, (h % 2) * HD
                ps_o = psO.tile([VW, LQ], F32)
                for kc2 in range(16):
                    ps_s = psS.tile([P, 1024], F32)
                    for j in range(2):
                        kc = 2 * kc2 + j
                        mm(
                            ps_s[:, j * LQ : (j + 1) * LQ],
                            kT4[off : off + HD, hl // 2, kc, :],
                            qT_all[off : off + HD, hp, :],
                            start=True,
                            stop=True,
                        )
                    ptile = ppool.tile([P, 1024], BF16)
                    act(ptile[:], ps_s[:], AF.Exp, scale=0.125)
                    for j in range(2):
                        kc = 2 * kc2 + j
                        mm(
                            ps_o[:],
                            v_g[:, kc, hl * VW : (hl + 1) * VW],
                            ptile[:, j * LQ : (j + 1) * LQ],
                            start=(kc == 0),
                            stop=(kc == L // P - 1),
                        )
                zrow = tiny.tile([1, LQ], F32, tag="zrow")
                v.tensor_copy(zrow[:], ps_o[HD : HD + 1, :])
                rz = tiny.tile([1, LQ], F32, tag="rz")
                v.reciprocal_approx_fast(rz[:], zrow[:])
                rz_b = rzb_pool.tile([P, LQ], F32)
                nc.gpsimd.partition_broadcast(rz_b[:], rz[:])
                v.tensor_tensor(
                    catT_all[off : off + HD, hp, :],
                    ps_o[0:HD, :],
                    rz_b[0:HD, :],
                    op=OP.mult,
                )

    # FFN2 weights land while the out-projection/LN2/FFN1 run (kv space freed)
    w2pool = ctx.enter_context(tc.tile_pool(name="wF2", bufs=1))
    wf2_sb = w2pool.tile([P, NMC, D], BF16)
    dma(out=wf2_sb[:], in_=t["w_ffn2"].rearrange("(c p) m -> p c m", p=P))
    bf2_b = w2pool.tile([P, D], F32)
    dma(out=bf2_b[:], in_=t["b_ffn2_b"][:])

    # -------- phase D: out-projection, gate, residual, LN2 ----------------
    with ExitStack() as phD:
        pool = phD.enter_context(tc.tile_pool(name="phD", bufs=2))
        spool = phD.enter_context(tc.tile_pool(name="spoolD", bufs=3))
        nxpool = phD.enter_context(tc.tile_pool(name="nxD", bufs=2))
        psD1 = phD.enter_context(tc.tile_pool(name="psD1", bufs=2, space="PSUM"))
        psD2 = phD.enter_context(tc.tile_pool(name="psD2", bufs=2, space="PSUM"))
        psT = phD.enter_context(tc.tile_pool(name="psTD", bufs=2, space="PSUM"))

        for qc in range(NQC):
            ps1 = psD1.tile([P, 512], F32)
            ps2 = psD2.tile([P, 256], F32)
            for cc in range(NDC):
                lhs = catT_all[:, cc, qc * P : (qc + 1) * P]
                mm(ps1[:], lhs, wao_sb[:, cc, 0:512],
                   start=(cc == 0), stop=(cc == NDC - 1))
                mm(ps2[:], lhs, wao_sb[:, cc, 512:768],
                   start=(cc == 0), stop=(cc == NDC - 1))
            yb = pool.tile([P, D], F32)
            v.tensor_add(yb[:, 0:512], ps1[:], ba_sb[:, 0:512])
            v.tensor_add(yb[:, 512:768], ps2[:], ba_sb[:, 512:768])
            yg = pool.tile([P, D], F32)
            v.tensor_tensor(yg[:], yb[:], g1_b[:], op=OP.mult)
            v.tensor_add(x2_loc[qc][:], yg[:], x_loc[:, qc, :])
            xq = x2_loc[qc][:]
            mv, rstd = _ln_rstd(nc, spool, eps_t, xq)
            nx = nxpool.tile([P, D], BF16)
            v.tensor_scalar(
                nx[:], xq, mv[:, 0:1], rstd[:], op0=OP.subtract, op1=OP.mult
            )
            for dc in range(NDC):
                pt = psT.tile([P, P], BF16)
                nc.tensor.transpose(pt[:], nx[:, dc * P : (dc + 1) * P], identity[:])
                act(
                    xn2T[:, dc, qc * P : (qc + 1) * P],
                    pt[:],
                    AF.Identity,
                    bias=sh2[:, dc : dc + 1],
                    scale=sp2[:, dc : dc + 1],
                )
    # ---------------- phase F: FFN + gate + residual -> output -------------
    with ExitStack() as phF:
        hpool = phF.enter_context(tc.tile_pool(name="hT", bufs=1))
        pool = phF.enter_context(tc.tile_pool(name="phF", bufs=2))
        psF1 = phF.enter_context(tc.tile_pool(name="psF1", bufs=3, space="PSUM"))
        psF2 = phF.enter_context(tc.tile_pool(name="psF2", bufs=2, space="PSUM"))

        hT_all = hpool.tile([P, NMC, LQ], BF16)
        for mc in range(NMC):
            ps_h = psF1.tile([P, LQ], F32, tag="mm512")
            for dc in range(NDC):
                mm(
                    ps_h[:],
                    wf1_sb[:, dc, mc * P : (mc + 1) * P],
                    xn2T[:, dc, :],
                    start=(dc == 0),
                    stop=(dc == NDC - 1),
                )
            act(hT_all[:, mc, :], ps_h[:], AF.Gelu, bias=bf1_col[:, mc : mc + 1])

        out_r = t["out"].rearrange("(n p) d -> n p d", p=P)
        for qc in range(NQC):
            ps1 = psF1.tile([P, 512], F32, tag="mm512")
            ps2 = psF2.tile([P, 256], F32)
            for mc in range(NMC):
                lhs = hT_all[:, mc, qc * P : (qc + 1) * P]
                mm(ps1[:], lhs, wf2_sb[:, mc, 0:512],
                   start=(mc == 0), stop=(mc == NMC - 1))
                mm(ps2[:], lhs, wf2_sb[:, mc, 512:768],
                   start=(mc == 0), stop=(mc == NMC - 1))
            y2 = pool.tile([P, D], F32)
            v.tensor_add(y2[:, 0:512], ps1[:], bf2_b[:, 0:512])
            v.tensor_add(y2[:, 512:768], ps2[:], bf2_b[:, 512:768])
            yg = pool.tile([P, D], F32)
            v.tensor_tensor(yg[:], y2[:], g2_b[:], op=OP.mult)
            ot = pool.tile([P, D], F32)
            v.tensor_add(ot[:], yg[:], x2_loc[qc][:])
            dma(out=out_r[qc], in_=ot[:])


def build_nc():
    nc = bacc.Bacc(
        None, target_bir_lowering=False, debug=False, num_devices=NCORES
    )
    t = _declare_params(nc)
    with tile.TileContext(nc) as tc:
        with ExitStack() as ctx:
            _build_body(nc, tc, ctx, t)
    nc.compile()
    return nc


_cache = {}


def _prep_in_maps(inputs):
    bf = lambda a: np.ascontiguousarray(np.asarray(a, np.float32)).astype(
        ml_dtypes.bfloat16
    )
    f32 = lambda a: np.ascontiguousarray(np.asarray(a, np.float32))
    x = f32(inputs["x"]).reshape(L, D)
    cond = f32(inputs["cond"]).reshape(D)
    b_qkv = f32(inputs["b_qkv"]).reshape(3 * D)
    b_adaln1 = f32(inputs["b_adaln1"]).reshape(3 * D)
    b_adaln2 = f32(inputs["b_adaln2"]).reshape(3 * D)
    common = {
        "cond_t": np.ascontiguousarray(cond.reshape(NDC, P).T),
        "w_adaln1": bf(inputs["w_adaln1"]),
        "w_adaln2": bf(inputs["w_adaln2"]),
        "b_adaln1_col": np.ascontiguousarray(b_adaln1[: 12 * P].reshape(12, P).T),
        "b_adaln2_col": np.ascontiguousarray(b_adaln2[: 12 * P].reshape(12, P).T),
        "b_adaln1_gate": np.ascontiguousarray(b_adaln1[2 * D :][None]),
        "b_adaln2_gate": np.ascontiguousarray(b_adaln2[2 * D :][None]),
        "w_qkv": bf(inputs["w_qkv"]),
        "bq_col": np.ascontiguousarray(b_qkv[:D].reshape(NHP, P).T),
        "bk_col": np.ascontiguousarray(b_qkv[D : 2 * D].reshape(NHP, P).T),
        "bv_b": np.ascontiguousarray(np.broadcast_to(b_qkv[2 * D :], (P, D))),
        "w_attn_out": bf(inputs["w_attn_out"]),
        "b_attn_b": np.ascontiguousarray(
            np.broadcast_to(f32(inputs["b_attn_out"]).reshape(D), (P, D))
        ),
        "w_ffn1": bf(inputs["w_ffn1"]),
        "b_ffn1_col": np.ascontiguousarray(
            f32(inputs["b_ffn1"]).reshape(NMC, P).T
        ),
        "w_ffn2": bf(inputs["w_ffn2"]),
        "b_ffn2_b": np.ascontiguousarray(
            np.broadcast_to(f32(inputs["b_ffn2"]).reshape(D), (P, D))
        ),
    }
    in_maps = []
    for c in range(NCORES):
        m = dict(common)
        m["x_loc"] = np.ascontiguousarray(x[c * LQ : (c + 1) * LQ])
        in_maps.append(m)
    return in_maps


def kernel(**inputs):
    if "nc" not in _cache:
        _cache["nc"] = build_nc()
    nc = _cache["nc"]
    in_maps = _prep_in_maps(inputs)
    res = run_bass_kernel_spmd(nc, in_maps, list(range(NCORES)))
    out = np.concatenate([res.results[c]["out"] for c in range(NCORES)], axis=0)
    return out.reshape(1, L, D).astype(np.float32)


if __name__ == "__main__":
    rng = np.random.default_rng(0)
    fake = {
        "x": rng.standard_normal((1, L, D), dtype=np.float32),
        "cond": rng.standard_normal((1, D), dtype=np.float32),
        "w_adaln1": rng.standard_normal((D, 3 * D), dtype=np.float32) * 0.02,
        "b_adaln1": np.zeros(3 * D, np.float32),
        "w_qkv": rng.standard_normal((D, 3 * D), dtype=np.float32) * D**-0.5,
        "b_qkv": np.zeros(3 * D, np.float32),
        "w_attn_out": rng.standard_normal((D, D), dtype=np.float32) * D**-0.5,
        "b_attn_out": np.zeros(D, np.float32),
        "w_adaln2": rng.standard_normal((D, 3 * D), dtype=np.float32) * 0.02,
        "b_adaln2": np.zeros(3 * D, np.float32),
        "w_ffn1": rng.standard_normal((D, DM), dtype=np.float32) * D**-0.5,
        "b_ffn1": np.zeros(DM, np.float32),
        "w_ffn2": rng.standard_normal((DM, D), dtype=np.float32) * DM**-0.5,
        "b_ffn2": np.zeros(D, np.float32),
    }
    out = kernel(**fake)
    print("out", out.shape, out.dtype, np.abs(out).max())
